# revision 47
# baseline (speedup 1.0000x reference)
"""Equivariant MPNN layer as a Bass/Tile kernel for TRN2.

Strategy (per problem nn_EquivariantMPNNLayer):
  - Edges are sorted by destination grid cell (j) on the host and sharded
    across cores by contiguous 128-segment blocks (G segs / NCORES per core).
  - Per-edge node embeddings are pre-ordered on the host (pure data
    movement) into nembET [H, e_pad] bf16 and streamed in large chunk DMAs;
    their @ Wm1_top contribution accumulates into the same PSUM tile as the
    geometry path (no per-edge gather on device).
  - Geometry: zin [21, e_pad] bf16 rows = [gp_rep(9) | R(9) | rp(3)];
    an in-place DVE mult forms z[9:18] = R*gp so mm1's rhs is the
    contiguous rows 9:21 with W1big = [repeat(We1,3); -We1].
  - msg' = silu(silu(pre1+be1) @ Wf + nembE @ Wm1top [+ bmix]) with
    Wf = We2 @ Wm1_bot (Wm2 folded out of the edge path). The bmix
    constant (bm1 + be2@Wm1_bot) is zero for this problem's inputs; the
    add is emitted only when nonzero.
  - Segment sums via one-hot matmul into PSUM per 128-seg block
    (lhsT=A one-hot [e,seg], rhs=msg'); counts come from the host.
  - Post: mean = (S/max(cnt,1)) @ Wm2 + bm2*[cnt>0], out = MLP_u(mean),
    all in [h, seg] layout with stationary weights.
Outputs are [128, SEGS_PER_CORE] (transposed) per core; host reassembles.
"""

import math
from contextlib import ExitStack

import numpy as np
import ml_dtypes

import concourse.bass as bass
import concourse.tile as tile
from concourse import bacc, mybir

F32 = mybir.dt.float32
BF16 = mybir.dt.bfloat16
AOT = mybir.AluOpType

H = 128
P = 128
BLK = 128  # segments per psum block


class Cfg:
    def __init__(self, N, G, E, B, ncores, T, reps=1, has_bmix=False,
                 sim_silu=False, loop_k=1, probe=()):
        self.sim_silu = sim_silu
        self.probe = set(probe)   # timing-only experiments, wrong numerics
        self.loop_k = loop_k
        self.N, self.G, self.E, self.B = N, G, E, B
        self.ncores = ncores
        assert G % (ncores * BLK) == 0
        self.segs_core = G // ncores          # segments per core
        self.nblk = self.segs_core // BLK     # psum blocks per core
        self.T = T                            # tiles (of 128 edges) per block
        self.ntiles = self.nblk * T           # tiles per core
        self.e_pad = self.ntiles * P          # padded edges per core
        self.chunk_tiles = 32                 # tiles per dma chunk
        assert self.ntiles % self.chunk_tiles == 0
        self.nchunks = self.ntiles // self.chunk_tiles
        self.chunk_e = self.chunk_tiles * P   # 8192
        self.reps = reps
        self.has_bmix = has_bmix


def _silu(nc, cfg, pool, out_ap, in_ap, bias_sb=None, tag="sig"):
    """out = silu(in_ + bias). Real Silu on HW; Sigmoid+DVE mult in sim."""
    if "noact" in cfg.probe:
        nc.vector.tensor_copy(out=out_ap, in_=in_ap)
        return
    if not cfg.sim_silu:
        if bias_sb is not None:
            nc.scalar.activation(out=out_ap, in_=in_ap,
                                 func=mybir.ActivationFunctionType.Silu,
                                 bias=bias_sb[:])
        else:
            nc.scalar.activation(out=out_ap, in_=in_ap,
                                 func=mybir.ActivationFunctionType.Silu)
        return
    sig = pool.tile(list(out_ap.shape), F32, tag=tag)
    if bias_sb is not None:
        nc.scalar.activation(out=sig[:], in_=in_ap,
                             func=mybir.ActivationFunctionType.Sigmoid,
                             bias=bias_sb[:])
        nc.vector.scalar_tensor_tensor(out=out_ap, in0=in_ap,
                                       scalar=bias_sb[:, :1], in1=sig[:],
                                       op0=AOT.add, op1=AOT.mult)
    else:
        nc.scalar.activation(out=sig[:], in_=in_ap,
                             func=mybir.ActivationFunctionType.Sigmoid)
        nc.vector.scalar_tensor_tensor(out=out_ap, in0=in_ap, scalar=0.0,
                                       in1=sig[:], op0=AOT.add, op1=AOT.mult)


def build_program(cfg: Cfg):
    """Build the SPMD per-core Bass program. Returns compiled nc."""
    nc = bacc.Bacc("TRN2", target_bir_lowering=False, debug=False,
                   num_devices=cfg.ncores)

    # ---------------- I/O ----------------
    def din(name, shape, dt=F32):
        return nc.dram_tensor(name, shape, dt, kind="ExternalInput").ap()

    W1bigb = din("W1bigb", [12, H], BF16)             # [repeat(We1,3); -We1]
    be1c = din("be1c", [H, 1])
    Wfb = din("Wfb", [H, H], BF16)                    # We2 @ Wm1_bot
    Wm1topb = din("Wm1topb", [H, H], BF16)            # Wm1[:H]
    I128 = din("I128", [P, P])                        # identity fp32
    IOTA = din("IOTA", [P, P], BF16)                  # IOTA[e,s] = s
    Wm2 = din("Wm2", [H, H])
    bm2r = din("bm2r", [1, H])
    Wu1 = din("Wu1", [H, H])
    bu1c = din("bu1c", [H, 1])
    Wu2 = din("Wu2", [H, H])
    bu2c = din("bu2c", [H, 1])
    if cfg.has_bmix:
        bmix4 = din("bmix4", [P, 512])                # bmix bcast rows, 4x
    if cfg.loop_k > 1:
        din("ktag", [1, cfg.loop_k])                  # shape tag vs HLO cache

    zin = din("zin", [21, cfg.e_pad], BF16)           # [gp_rep; R; rp]
    nembET = din("nembET", [H, cfg.e_pad], BF16)      # node_emb.T per edge
    segf = din("segf", [P, cfg.ntiles], F32)          # seg-in-block (or -1)
    rinv = din("rinv", [P, cfg.nblk])                 # 1/max(cnt,1) per seg
    gates = din("gates", [1, cfg.segs_core])          # min(cnt,1) per seg

    outT = nc.dram_tensor("outT", [H, cfg.segs_core], F32,
                          kind="ExternalOutput").ap()

    with tile.TileContext(nc) as tc, ExitStack() as ctx:
        ep = ctx.enter_context  # shorthand

        consts = ep(tc.tile_pool(name="consts", bufs=1))
        gpool = ep(tc.tile_pool(name="gpool", bufs=4))
        zpool = ep(tc.tile_pool(name="zpool", bufs=4))
        epool = ep(tc.tile_pool(name="epool", bufs=8))
        postp = ep(tc.tile_pool(name="postp", bufs=2))
        simp = ep(tc.tile_pool(name="simp", bufs=2))
        apool = ep(tc.tile_pool(name="apool", bufs=16))
        spool = ep(tc.tile_pool(name="spool", bufs=1))
        ppool = ep(tc.tile_pool(name="ppool", bufs=3, space="PSUM"))
        pm2 = ep(tc.tile_pool(name="pm2", bufs=3, space="PSUM"))
        psS = ep(tc.tile_pool(name="psS", bufs=2, space="PSUM"))
        psT = psS

        # ---- load constants into SBUF ----
        def csb(ap_in, shape, dt=F32, tag=None):
            t = consts.tile(shape, dt, tag=tag or ap_in.tensor.name)
            nc.sync.dma_start(t[:], ap_in)
            return t

        W1big_sb = csb(W1bigb, [12, H], BF16)
        be1_sb = csb(be1c, [H, 1])
        Wf_sb = csb(Wfb, [H, H], BF16)
        Wm1top_sb = csb(Wm1topb, [H, H], BF16)
        I128_sb = csb(I128, [P, P])
        IOTA_sb = csb(IOTA, [P, P], BF16)
        Wm2_sb = csb(Wm2, [H, H])
        bm2_sb = csb(bm2r, [1, H])
        Wu1_sb = csb(Wu1, [H, H])
        bu1_sb = csb(bu1c, [H, 1])
        Wu2_sb = csb(Wu2, [H, H])
        bu2_sb = csb(bu2c, [H, 1])
        bmix_sb = csb(bmix4, [P, 512]) if cfg.has_bmix else None
        segf_sb = consts.tile([P, cfg.ntiles], F32, tag="segf")
        nc.sync.dma_start(segf_sb[:], segf)
        rinv_sb = consts.tile([P, cfg.nblk], F32, tag="rinv")
        nc.sync.dma_start(rinv_sb[:], rinv)
        gflat = consts.tile([1, cfg.segs_core], F32, tag="gates")
        nc.sync.dma_start(gflat[:], gates)

        # ---- main edge phase (repeatable for timing) ----
        loop_cm = tc.For_i(0, cfg.loop_k, 1) if cfg.loop_k > 1 else None
        if loop_cm is not None:
            ctx.enter_context(loop_cm)
        for rep in range(cfg.reps):
            ST_all = spool.tile([H, cfg.segs_core], F32, tag="ST")
            state = {"ps_blk": None}
            pending = None  # (group_base_tile, msgp tile) delayed one group

            def emit_scatter(gbase, msgp):
                for t8 in range(4):
                    t = gbase + t8
                    blk = t // cfg.T
                    tin = t % cfg.T
                    if "noscat" in cfg.probe and tin not in (0, cfg.T - 1):
                        continue
                    if "nodve" in cfg.probe:
                        at = IOTA_sb
                    else:
                        at = apool.tile([P, P], BF16, tag="at")
                        nc.vector.tensor_scalar(
                            out=at[:], in0=IOTA_sb[:],
                            scalar1=segf_sb[:, t:t + 1], scalar2=None,
                            op0=AOT.is_equal)
                    if tin == 0:
                        ps_new = psS.tile([P, H], F32, tag="psS")
                        state["ps_blk"] = ps_new
                    nc.tensor.matmul(out=state["ps_blk"][:], lhsT=at[:],
                                     rhs=msgp[:, t8 * 128:t8 * 128 + 128],
                                     start=(tin == 0),
                                     stop=(tin == cfg.T - 1))
                    if tin == cfg.T - 1:
                        _finish_block(nc, tc, cfg, blk, state["ps_blk"],
                                      rinv_sb, I128_sb, ST_all, apool, psT)

            z_once = None
            ec_once = None
            for c in range(cfg.nchunks):
                e0 = c * cfg.chunk_e
                if "onez" in cfg.probe and z_once is not None:
                    zg, zr = z_once
                else:
                    zg = zpool.tile([9, cfg.chunk_e], BF16, tag="zg")
                    nc.sync.dma_start(zg[:], zin[0:9, e0:e0 + cfg.chunk_e])
                    zr = zpool.tile([12, cfg.chunk_e], BF16, tag="zr")
                    nc.sync.dma_start(zr[:], zin[9:21, e0:e0 + cfg.chunk_e])
                    z_once = (zg, zr)
                if "oneec" in cfg.probe and ec_once is not None:
                    ec = ec_once
                else:
                    ec = gpool.tile([H, cfg.chunk_e], BF16, tag="ec")
                    if "halfdma" in cfg.probe:
                        nc.sync.dma_start(ec[0:64, :],
                                          nembET[0:64, e0:e0 + cfg.chunk_e])
                    else:
                        nc.sync.dma_start(ec[:],
                                          nembET[:, e0:e0 + cfg.chunk_e])
                    ec_once = ec
                for s in range(cfg.chunk_tiles // 4):  # 512-edge groups
                    g0 = s * 512
                    gs = slice(g0, g0 + 512)
                    # zr[0:9] = gp_rep * R (in place); zr[9:12] stays rp
                    nc.vector.tensor_tensor(out=zr[0:9, gs],
                                            in0=zg[0:9, gs],
                                            in1=zr[0:9, gs], op=AOT.mult)
                    pp = ppool.tile([H, 512], F32, tag="pre1")
                    nc.tensor.matmul(out=pp[:], lhsT=W1big_sb[:],
                                     rhs=zr[0:12, gs],
                                     start=True, stop=True)
                    h1 = epool.tile([H, 512], BF16, tag="h1")
                    _silu(nc, cfg, simp, h1[:], pp[:], be1_sb, tag="sig1")
                    if pending is not None:
                        emit_scatter(*pending)
                        pending = None
                    pm = pm2.tile([P, 512], F32, tag="pm")
                    for t4 in range(4):
                        co = t4 * 128
                        one_mm = "nope" in cfg.probe
                        nc.tensor.matmul(out=pm[:, co:co + 128],
                                         lhsT=h1[:, co:co + 128], rhs=Wf_sb[:],
                                         start=True, stop=one_mm)
                        if not one_mm:
                            nc.tensor.matmul(out=pm[:, co:co + 128],
                                             lhsT=ec[:, g0 + co:g0 + co + 128],
                                             rhs=Wm1top_sb[:],
                                             start=False, stop=True)
                    if cfg.has_bmix:
                        nc.vector.tensor_tensor(out=pm[:], in0=pm[:],
                                                in1=bmix_sb[:], op=AOT.add)
                    msgp = epool.tile([P, 512], BF16, tag="msgp")
                    _silu(nc, cfg, simp, msgp[:], pm[:], None, tag="sig2")
                    pending = (c * cfg.chunk_tiles + s * 4, msgp)

            if pending is not None:
                emit_scatter(*pending)
                pending = None

            _post_stage(nc, tc, cfg, ST_all, gflat, Wm2_sb, bm2_sb, Wu1_sb,
                        bu1_sb, Wu2_sb, bu2_sb, I128_sb, outT,
                        apool, postp, psT, pm2, ppool, simp)

    nc.compile()
    return nc


def _finish_block(nc, tc, cfg, blk, ps_blk, rinv_sb, I128_sb, ST_all,
                  apool, psT):
    """Scale block's psum by 1/max(cnt,1) and transpose into ST_all."""
    sp = apool.tile([P, H], F32, tag="sprime")
    nc.vector.tensor_scalar(out=sp[:], in0=ps_blk[:],
                            scalar1=rinv_sb[:, blk:blk + 1],
                            scalar2=None, op0=AOT.mult)
    pt = psT.tile([P, H], F32, tag="psS")
    nc.tensor.transpose(out=pt[:], in_=sp[:], identity=I128_sb[:])
    nc.vector.tensor_copy(out=ST_all[:, blk * BLK:(blk + 1) * BLK], in_=pt[:])


def _post_stage(nc, tc, cfg, ST_all, gflat, Wm2_sb, bm2_sb, Wu1_sb, bu1_sb,
                Wu2_sb, bu2_sb, I128_sb, outT, apool, epool, psT, pm2, ppool,
                simp=None):

    nseg_chunks = math.ceil(cfg.segs_core / 512)
    for u in range(nseg_chunks):
        s0 = u * 512
        w = min(512, cfg.segs_core - s0)
        pmm = ppool.tile([H, 512], F32, tag="pre1")
        nc.tensor.matmul(out=pmm[:, :w], lhsT=Wm2_sb[:],
                         rhs=ST_all[:, s0:s0 + w], start=True, stop=False)
        nc.tensor.matmul(out=pmm[:, :w], lhsT=bm2_sb[:],
                         rhs=gflat[0:1, s0:s0 + w], start=False, stop=True)
        mean = epool.tile([H, 512], F32, tag="mean")
        nc.vector.tensor_copy(out=mean[:, :w], in_=pmm[:, :w])
        pu = pm2.tile([H, 512], F32, tag="pm")
        nc.tensor.matmul(out=pu[:, :w], lhsT=Wu1_sb[:], rhs=mean[:, :w],
                         start=True, stop=True)
        hu = epool.tile([H, 512], F32, tag="hu")
        _silu(nc, cfg, simp or epool, hu[:, :w], pu[:, :w], bu1_sb, tag="sigu")
        po = ppool.tile([H, 512], F32, tag="pre1")
        nc.tensor.matmul(out=po[:, :w], lhsT=Wu2_sb[:], rhs=hu[:, :w],
                         start=True, stop=True)
        ot = epool.tile([H, 512], F32, tag="ot")
        nc.vector.tensor_scalar(out=ot[:, :w], in0=po[:, :w],
                                scalar1=bu2_sb[:, :1], scalar2=None,
                                op0=AOT.add)
        nc.sync.dma_start(outT[:, s0:s0 + w], ot[:, :w])


# ======================= host preprocessing =======================

def silu_np(x):
    return x / (1.0 + np.exp(-x))


def host_prep(inputs, ncores, t_override=None):
    """Returns (cfg, list of per-core in_maps, const row for node outputs)."""
    nemb = np.asarray(inputs["node_embedding"], np.float32)
    npos = np.asarray(inputs["node_pos"], np.float32)
    gpos = np.asarray(inputs["grid_pos"], np.float32)
    eidx = np.asarray(inputs["edge_index"], np.int64)
    frames = np.asarray(inputs["equi_frames"], np.float32)
    batch = np.asarray(inputs["batch"], np.int64)
    We1 = np.asarray(inputs["We1"], np.float32); be1 = np.asarray(inputs["be1"], np.float32)
    We2 = np.asarray(inputs["We2"], np.float32); be2 = np.asarray(inputs["be2"], np.float32)
    Wm1 = np.asarray(inputs["Wm1"], np.float32); bm1 = np.asarray(inputs["bm1"], np.float32)
    Wm2 = np.asarray(inputs["Wm2"], np.float32); bm2 = np.asarray(inputs["bm2"], np.float32)
    Wu1 = np.asarray(inputs["Wu1"], np.float32); bu1 = np.asarray(inputs["bu1"], np.float32)
    Wu2 = np.asarray(inputs["Wu2"], np.float32); bu2 = np.asarray(inputs["bu2"], np.float32)

    N, Hh = nemb.shape
    G = gpos.shape[0]
    E = eidx.shape[1]
    B = frames.shape[0]
    assert Hh == H

    i_all = eidx[0]
    jg_all = eidx[1] - N
    order = np.argsort(jg_all, kind="stable")
    jg_s = jg_all[order]
    i_s = i_all[order]

    gb = jg_s // BLK
    counts_blk = np.bincount(gb, minlength=G // BLK)
    T = int(math.ceil(counts_blk.max() / P))
    if t_override:
        T = max(T, t_override)
    bmix = bm1 + be2 @ Wm1[H:]
    has_bmix = bool(np.any(np.abs(bmix) > 0))
    cfg = Cfg(N, G, E, B, ncores, T, has_bmix=has_bmix)

    # destination slot for each sorted edge
    starts = np.zeros(G // BLK + 1, np.int64)
    starts[1:] = np.cumsum(counts_blk)
    rank = np.arange(E) - starts[gb]
    core_e = gb // cfg.nblk
    b_local = gb % cfg.nblk
    slot = b_local * (T * P) + rank

    # per-edge host gathers (pure data movement + O(N)/O(B) math)
    R_flat = frames.reshape(B, 9)
    b_e = batch[i_s]
    z_r = R_flat[b_e].T.astype(np.float32)               # [9, E]
    gp_e = gpos[jg_s].T.astype(np.float32)               # [3, E]
    gp_rep = np.tile(gp_e, (3, 1))                       # [9, E]
    rp_node = np.einsum("nab,nb->na", frames[batch], npos).astype(np.float32)
    rp_e = rp_node[i_s].T                                # [3, E]

    ecount = np.bincount(jg_all, minlength=G).astype(np.float32)
    nemb_bf = nemb.astype(ml_dtypes.bfloat16)

    # shared constant tensors
    shared = {
        "W1bigb": np.ascontiguousarray(
            np.concatenate([np.repeat(We1, 3, axis=0), -We1], 0)
        ).astype(ml_dtypes.bfloat16),
        "be1c": np.ascontiguousarray(be1[:, None]),
        "Wfb": np.ascontiguousarray(We2 @ Wm1[H:]).astype(ml_dtypes.bfloat16),
        "Wm1topb": np.ascontiguousarray(Wm1[:H]).astype(ml_dtypes.bfloat16),
        "I128": np.eye(P, dtype=np.float32),
        "IOTA": np.ascontiguousarray(
            np.tile(np.arange(P, dtype=np.float32)[None, :], (P, 1))
        ).astype(ml_dtypes.bfloat16),
        "Wm2": np.ascontiguousarray(Wm2),
        "bm2r": np.ascontiguousarray(bm2[None, :]),
        "Wu1": np.ascontiguousarray(Wu1),
        "bu1c": np.ascontiguousarray(bu1[:, None]),
        "Wu2": np.ascontiguousarray(Wu2),
        "bu2c": np.ascontiguousarray(bu2[:, None]),
    }
    if has_bmix:
        shared["bmix4"] = np.ascontiguousarray(
            np.tile(bmix[None, :], (P, 4)))

    in_maps = []
    for c in range(ncores):
        sel = core_e == c
        sl = slot[sel]
        zin = np.zeros((21, cfg.e_pad), np.float32)
        zin[0:9, sl] = gp_rep[:, sel]
        zin[9:18, sl] = z_r[:, sel]
        zin[18:21, sl] = rp_e[:, sel]
        embT = np.zeros((cfg.e_pad, H), ml_dtypes.bfloat16)
        embT[sl] = nemb_bf[i_s[sel]]
        segf_f = np.full(cfg.e_pad, -1.0, np.float32)
        segf_f[sl] = (jg_s[sel] % BLK).astype(np.float32)

        segf_t = segf_f.reshape(cfg.ntiles, P).T            # [128, ntiles]
        cnt_c = ecount[c * cfg.segs_core:(c + 1) * cfg.segs_core]
        rinv_t = (1.0 / np.maximum(cnt_c, 1.0)) \
            .reshape(cfg.nblk, P).T.astype(np.float32)      # [128, nblk]

        m = dict(shared)
        m["zin"] = np.ascontiguousarray(zin.astype(ml_dtypes.bfloat16))
        m["nembET"] = np.ascontiguousarray(embT.T)
        m["segf"] = np.ascontiguousarray(segf_t)
        m["rinv"] = np.ascontiguousarray(rinv_t)
        m["gates"] = np.ascontiguousarray(
            np.minimum(cnt_c, 1.0)[None, :].astype(np.float32))
        in_maps.append(m)

    const_row = silu_np(bu1) @ Wu2 + bu2
    return cfg, in_maps, const_row


def assemble_output(cfg, results, const_row, N, G):
    out = np.empty((N + G, H), np.float32)
    out[:N] = const_row[None, :]
    for c in range(cfg.ncores):
        out[N + c * cfg.segs_core: N + (c + 1) * cfg.segs_core] = \
            results[c]["outT"].T
    return out


# ======================= top-level kernel entry =======================

_PROGRAM_CACHE = {}

NCORES = 8


def kernel(**inputs):
    """Full-input entry point: shards edges by destination grid cell across
    8 NeuronCores, runs the Bass/Tile program, reassembles the full output."""
    from concourse.bass_utils import run_bass_kernel_spmd

    cfg, in_maps, const_row = host_prep(inputs, NCORES)
    key = (cfg.T, cfg.has_bmix)
    if key not in _PROGRAM_CACHE:
        _PROGRAM_CACHE[key] = build_program(cfg)
    nc = _PROGRAM_CACHE[key]
    res = run_bass_kernel_spmd(nc, in_maps, core_ids=list(range(NCORES)))
    N = inputs["node_pos"].shape[0]
    G = inputs["grid_pos"].shape[0]
    return assemble_output(cfg, res.results, const_row, N, G)


# revision 57
# speedup vs baseline: 1.3062x; 1.3062x over previous
"""Equivariant MPNN layer as a Bass/Tile kernel for TRN2.

Strategy (per problem nn_EquivariantMPNNLayer):
  - Edges are sorted by destination grid cell (j) on the host and sharded
    across cores by contiguous 128-segment blocks (G segs / NCORES per core).
  - Per-edge node embeddings are pre-ordered on the host (pure data
    movement) into nembET [H, e_pad] bf16 and streamed in large chunk DMAs;
    their @ Wm1_top contribution accumulates into the same PSUM tile as the
    geometry path (no per-edge gather on device).
  - Geometry: zin [21, e_pad] bf16 rows = [gp_rep(9) | R(9) | rp(3)];
    an in-place DVE mult forms z[9:18] = R*gp so mm1's rhs is the
    contiguous rows 9:21 with W1big = [repeat(We1,3); -We1].
  - msg' = silu(silu(pre1+be1) @ Wf + nembE @ Wm1top [+ bmix]) with
    Wf = We2 @ Wm1_bot (Wm2 folded out of the edge path). The bmix
    constant (bm1 + be2@Wm1_bot) is zero for this problem's inputs; the
    add is emitted only when nonzero.
  - Segment sums via one-hot matmul into PSUM per 128-seg block
    (lhsT=A one-hot [e,seg], rhs=msg'); counts come from the host.
  - Post: mean = (S/max(cnt,1)) @ Wm2 + bm2*[cnt>0], out = MLP_u(mean),
    all in [h, seg] layout with stationary weights.
Outputs are [128, SEGS_PER_CORE] (transposed) per core; host reassembles.
"""

import math
from contextlib import ExitStack

import numpy as np
import ml_dtypes

import concourse.bass as bass
import concourse.tile as tile
from concourse import bacc, mybir

F32 = mybir.dt.float32
BF16 = mybir.dt.bfloat16
AOT = mybir.AluOpType

H = 128
P = 128
BLK = 128  # segments per psum block


class Cfg:
    def __init__(self, N, G, E, B, ncores, tiles_per_block, reps=1,
                 has_bmix=False, sim_silu=False, loop_k=1, probe=()):
        self.sim_silu = sim_silu
        self.probe = set(probe)   # timing-only experiments, wrong numerics
        self.loop_k = loop_k
        self.N, self.G, self.E, self.B = N, G, E, B
        self.ncores = ncores
        assert G % (ncores * BLK) == 0
        self.segs_core = G // ncores          # segments per core
        self.nblk = self.segs_core // BLK     # psum blocks per core
        self.tiles_per_block = tuple(tiles_per_block)   # len nblk, shared
        assert len(self.tiles_per_block) == self.nblk
        self.ntiles = sum(self.tiles_per_block)
        self.e_pad = self.ntiles * P          # padded edges per core
        self.chunk_tiles = 32                 # tiles per dma chunk
        assert self.ntiles % self.chunk_tiles == 0
        self.nchunks = self.ntiles // self.chunk_tiles
        self.chunk_e = self.chunk_tiles * P
        self.reps = reps
        self.has_bmix = has_bmix
        # per-tile metadata: owning program-block, first/last tile flags
        self.blk_of, self.t_first, self.t_last = [], [], []
        for i, nt in enumerate(self.tiles_per_block):
            for k in range(nt):
                self.blk_of.append(i)
                self.t_first.append(k == 0)
                self.t_last.append(k == nt - 1)


def _silu(nc, cfg, pool, out_ap, in_ap, bias_sb=None, tag="sig"):
    """out = silu(in_ + bias). Real Silu on HW; Sigmoid+DVE mult in sim."""
    if "noact" in cfg.probe:
        nc.vector.tensor_copy(out=out_ap, in_=in_ap)
        return
    if not cfg.sim_silu:
        if bias_sb is not None:
            nc.scalar.activation(out=out_ap, in_=in_ap,
                                 func=mybir.ActivationFunctionType.Silu,
                                 bias=bias_sb[:])
        else:
            nc.scalar.activation(out=out_ap, in_=in_ap,
                                 func=mybir.ActivationFunctionType.Silu)
        return
    sig = pool.tile(list(out_ap.shape), F32, tag=tag)
    if bias_sb is not None:
        nc.scalar.activation(out=sig[:], in_=in_ap,
                             func=mybir.ActivationFunctionType.Sigmoid,
                             bias=bias_sb[:])
        nc.vector.scalar_tensor_tensor(out=out_ap, in0=in_ap,
                                       scalar=bias_sb[:, :1], in1=sig[:],
                                       op0=AOT.add, op1=AOT.mult)
    else:
        nc.scalar.activation(out=sig[:], in_=in_ap,
                             func=mybir.ActivationFunctionType.Sigmoid)
        nc.vector.scalar_tensor_tensor(out=out_ap, in0=in_ap, scalar=0.0,
                                       in1=sig[:], op0=AOT.add, op1=AOT.mult)


def build_program(cfg: Cfg):
    """Build the SPMD per-core Bass program. Returns compiled nc."""
    nc = bacc.Bacc("TRN2", target_bir_lowering=False, debug=False,
                   num_devices=cfg.ncores)

    # ---------------- I/O ----------------
    def din(name, shape, dt=F32):
        return nc.dram_tensor(name, shape, dt, kind="ExternalInput").ap()

    W1bigb = din("W1bigb", [12, H], BF16)             # [repeat(We1,3); -We1]
    be1c = din("be1c", [H, 1])
    Wfb = din("Wfb", [H, H], BF16)                    # We2 @ Wm1_bot
    Wm1topb = din("Wm1topb", [H, H], BF16)            # Wm1[:H]
    I128 = din("I128", [P, P])                        # identity fp32
    IOTA = din("IOTA", [P, P], BF16)                  # IOTA[e,s] = s
    Wm2 = din("Wm2", [H, H])
    bm2r = din("bm2r", [1, H])
    Wu1 = din("Wu1", [H, H])
    bu1c = din("bu1c", [H, 1])
    Wu2 = din("Wu2", [H, H])
    bu2c = din("bu2c", [H, 1])
    if cfg.has_bmix:
        bmix4 = din("bmix4", [P, 512])                # bmix bcast rows, 4x
    if cfg.loop_k > 1:
        din("ktag", [1, cfg.loop_k])                  # shape tag vs HLO cache

    zin = din("zin", [21, cfg.e_pad], BF16)           # [gp_rep; R; rp]
    nembET = din("nembET", [H, cfg.e_pad], BF16)      # node_emb.T per edge
    segf = din("segf", [P, cfg.ntiles], F32)          # seg-in-block (or -1)
    rinv = din("rinv", [P, cfg.nblk])                 # 1/max(cnt,1) per seg
    gates = din("gates", [1, cfg.segs_core])          # min(cnt,1) per seg

    outT = nc.dram_tensor("outT", [H, cfg.segs_core], F32,
                          kind="ExternalOutput").ap()

    with tile.TileContext(nc) as tc, ExitStack() as ctx:
        ep = ctx.enter_context  # shorthand

        consts = ep(tc.tile_pool(name="consts", bufs=1))
        gpool = ep(tc.tile_pool(name="gpool", bufs=4))
        zpool = ep(tc.tile_pool(name="zpool", bufs=4))
        epool = ep(tc.tile_pool(name="epool", bufs=8))
        postp = ep(tc.tile_pool(name="postp", bufs=2))
        simp = ep(tc.tile_pool(name="simp", bufs=2))
        apool = ep(tc.tile_pool(name="apool", bufs=16))
        spool = ep(tc.tile_pool(name="spool", bufs=1))
        ppool = ep(tc.tile_pool(name="ppool", bufs=3, space="PSUM"))
        pm2 = ep(tc.tile_pool(name="pm2", bufs=3, space="PSUM"))
        psS = ep(tc.tile_pool(name="psS", bufs=2, space="PSUM"))
        psT = psS

        # ---- load constants into SBUF ----
        def csb(ap_in, shape, dt=F32, tag=None):
            t = consts.tile(shape, dt, tag=tag or ap_in.tensor.name)
            nc.sync.dma_start(t[:], ap_in)
            return t

        W1big_sb = csb(W1bigb, [12, H], BF16)
        be1_sb = csb(be1c, [H, 1])
        Wf_sb = csb(Wfb, [H, H], BF16)
        Wm1top_sb = csb(Wm1topb, [H, H], BF16)
        I128_sb = csb(I128, [P, P])
        IOTA_sb = csb(IOTA, [P, P], BF16)
        Wm2_sb = csb(Wm2, [H, H])
        bm2_sb = csb(bm2r, [1, H])
        Wu1_sb = csb(Wu1, [H, H])
        bu1_sb = csb(bu1c, [H, 1])
        Wu2_sb = csb(Wu2, [H, H])
        bu2_sb = csb(bu2c, [H, 1])
        bmix_sb = csb(bmix4, [P, 512]) if cfg.has_bmix else None
        segf_sb = consts.tile([P, cfg.ntiles], F32, tag="segf")
        nc.sync.dma_start(segf_sb[:], segf)
        rinv_sb = consts.tile([P, cfg.nblk], F32, tag="rinv")
        nc.sync.dma_start(rinv_sb[:], rinv)
        gflat = consts.tile([1, cfg.segs_core], F32, tag="gates")
        nc.sync.dma_start(gflat[:], gates)

        # ---- main edge phase (repeatable for timing) ----
        loop_cm = tc.For_i(0, cfg.loop_k, 1) if cfg.loop_k > 1 else None
        if loop_cm is not None:
            ctx.enter_context(loop_cm)
        for rep in range(cfg.reps):
            ST_all = spool.tile([H, cfg.segs_core], F32, tag="ST")
            state = {"ps_blk": None}
            pending = None  # (group_base_tile, msgp tile) delayed one group

            def emit_scatter(gbase, msgp):
                for t8 in range(4):
                    t = gbase + t8
                    blk = cfg.blk_of[t]
                    if "noscat" in cfg.probe and not (cfg.t_first[t]
                                                      or cfg.t_last[t]):
                        continue
                    if "nodve" in cfg.probe:
                        at = IOTA_sb
                    else:
                        at = apool.tile([P, P], BF16, tag="at")
                        nc.vector.tensor_scalar(
                            out=at[:], in0=IOTA_sb[:],
                            scalar1=segf_sb[:, t:t + 1], scalar2=None,
                            op0=AOT.is_equal)
                    if cfg.t_first[t]:
                        ps_new = psS.tile([P, H], F32, tag="psS")
                        state["ps_blk"] = ps_new
                    nc.tensor.matmul(out=state["ps_blk"][:], lhsT=at[:],
                                     rhs=msgp[:, t8 * 128:t8 * 128 + 128],
                                     start=cfg.t_first[t],
                                     stop=cfg.t_last[t])
                    if cfg.t_last[t]:
                        _finish_block(nc, tc, cfg, blk, state["ps_blk"],
                                      rinv_sb, I128_sb, ST_all, apool, psT)

            z_once = None
            ec_once = None
            for c in range(cfg.nchunks):
                e0 = c * cfg.chunk_e
                if "onez" in cfg.probe and z_once is not None:
                    zg, zr = z_once
                else:
                    zg = zpool.tile([9, cfg.chunk_e], BF16, tag="zg")
                    nc.sync.dma_start(zg[:], zin[0:9, e0:e0 + cfg.chunk_e])
                    zr = zpool.tile([12, cfg.chunk_e], BF16, tag="zr")
                    nc.sync.dma_start(zr[:], zin[9:21, e0:e0 + cfg.chunk_e])
                    z_once = (zg, zr)
                if "oneec" in cfg.probe and ec_once is not None:
                    ec = ec_once
                else:
                    ec = gpool.tile([H, cfg.chunk_e], BF16, tag="ec")
                    if "halfdma" in cfg.probe:
                        nc.sync.dma_start(ec[0:64, :],
                                          nembET[0:64, e0:e0 + cfg.chunk_e])
                    else:
                        nc.sync.dma_start(ec[:],
                                          nembET[:, e0:e0 + cfg.chunk_e])
                    ec_once = ec
                for s in range(cfg.chunk_tiles // 4):  # 512-edge groups
                    g0 = s * 512
                    gs = slice(g0, g0 + 512)
                    # zr[0:9] = gp_rep * R (in place); zr[9:12] stays rp
                    nc.vector.tensor_tensor(out=zr[0:9, gs],
                                            in0=zg[0:9, gs],
                                            in1=zr[0:9, gs], op=AOT.mult)
                    pp = ppool.tile([H, 512], F32, tag="pre1")
                    nc.tensor.matmul(out=pp[:], lhsT=W1big_sb[:],
                                     rhs=zr[0:12, gs],
                                     start=True, stop=True)
                    h1 = epool.tile([H, 512], BF16, tag="h1")
                    _silu(nc, cfg, simp, h1[:], pp[:], be1_sb, tag="sig1")
                    if pending is not None:
                        emit_scatter(*pending)
                        pending = None
                    pm = pm2.tile([P, 512], F32, tag="pm")
                    for t4 in range(4):
                        co = t4 * 128
                        one_mm = "nope" in cfg.probe
                        nc.tensor.matmul(out=pm[:, co:co + 128],
                                         lhsT=h1[:, co:co + 128], rhs=Wf_sb[:],
                                         start=True, stop=one_mm)
                        if not one_mm:
                            nc.tensor.matmul(out=pm[:, co:co + 128],
                                             lhsT=ec[:, g0 + co:g0 + co + 128],
                                             rhs=Wm1top_sb[:],
                                             start=False, stop=True)
                    if cfg.has_bmix:
                        nc.vector.tensor_tensor(out=pm[:], in0=pm[:],
                                                in1=bmix_sb[:], op=AOT.add)
                    msgp = epool.tile([P, 512], BF16, tag="msgp")
                    _silu(nc, cfg, simp, msgp[:], pm[:], None, tag="sig2")
                    pending = (c * cfg.chunk_tiles + s * 4, msgp)

            if pending is not None:
                emit_scatter(*pending)
                pending = None

            _post_stage(nc, tc, cfg, ST_all, gflat, Wm2_sb, bm2_sb, Wu1_sb,
                        bu1_sb, Wu2_sb, bu2_sb, I128_sb, outT,
                        apool, postp, psT, pm2, ppool, simp)

    nc.compile()
    return nc


def _finish_block(nc, tc, cfg, blk, ps_blk, rinv_sb, I128_sb, ST_all,
                  apool, psT):
    """Scale block's psum by 1/max(cnt,1) and transpose into ST_all."""
    sp = apool.tile([P, H], F32, tag="sprime")
    nc.vector.tensor_scalar(out=sp[:], in0=ps_blk[:],
                            scalar1=rinv_sb[:, blk:blk + 1],
                            scalar2=None, op0=AOT.mult)
    pt = psT.tile([P, H], F32, tag="psS")
    nc.tensor.transpose(out=pt[:], in_=sp[:], identity=I128_sb[:])
    nc.vector.tensor_copy(out=ST_all[:, blk * BLK:(blk + 1) * BLK], in_=pt[:])


def _post_stage(nc, tc, cfg, ST_all, gflat, Wm2_sb, bm2_sb, Wu1_sb, bu1_sb,
                Wu2_sb, bu2_sb, I128_sb, outT, apool, epool, psT, pm2, ppool,
                simp=None):

    nseg_chunks = math.ceil(cfg.segs_core / 512)
    for u in range(nseg_chunks):
        s0 = u * 512
        w = min(512, cfg.segs_core - s0)
        pmm = ppool.tile([H, 512], F32, tag="pre1")
        nc.tensor.matmul(out=pmm[:, :w], lhsT=Wm2_sb[:],
                         rhs=ST_all[:, s0:s0 + w], start=True, stop=False)
        nc.tensor.matmul(out=pmm[:, :w], lhsT=bm2_sb[:],
                         rhs=gflat[0:1, s0:s0 + w], start=False, stop=True)
        mean = epool.tile([H, 512], F32, tag="mean")
        nc.vector.tensor_copy(out=mean[:, :w], in_=pmm[:, :w])
        pu = pm2.tile([H, 512], F32, tag="pm")
        nc.tensor.matmul(out=pu[:, :w], lhsT=Wu1_sb[:], rhs=mean[:, :w],
                         start=True, stop=True)
        hu = epool.tile([H, 512], F32, tag="hu")
        _silu(nc, cfg, simp or epool, hu[:, :w], pu[:, :w], bu1_sb, tag="sigu")
        po = ppool.tile([H, 512], F32, tag="pre1")
        nc.tensor.matmul(out=po[:, :w], lhsT=Wu2_sb[:], rhs=hu[:, :w],
                         start=True, stop=True)
        ot = epool.tile([H, 512], F32, tag="ot")
        nc.vector.tensor_scalar(out=ot[:, :w], in0=po[:, :w],
                                scalar1=bu2_sb[:, :1], scalar2=None,
                                op0=AOT.add)
        nc.sync.dma_start(outT[:, s0:s0 + w], ot[:, :w])


# ======================= host preprocessing =======================

def silu_np(x):
    return x / (1.0 + np.exp(-x))


def host_prep(inputs, ncores, t_override=None):
    """Returns (cfg, list of per-core in_maps, const row for node outputs)."""
    nemb = np.asarray(inputs["node_embedding"], np.float32)
    npos = np.asarray(inputs["node_pos"], np.float32)
    gpos = np.asarray(inputs["grid_pos"], np.float32)
    eidx = np.asarray(inputs["edge_index"], np.int64)
    frames = np.asarray(inputs["equi_frames"], np.float32)
    batch = np.asarray(inputs["batch"], np.int64)
    We1 = np.asarray(inputs["We1"], np.float32); be1 = np.asarray(inputs["be1"], np.float32)
    We2 = np.asarray(inputs["We2"], np.float32); be2 = np.asarray(inputs["be2"], np.float32)
    Wm1 = np.asarray(inputs["Wm1"], np.float32); bm1 = np.asarray(inputs["bm1"], np.float32)
    Wm2 = np.asarray(inputs["Wm2"], np.float32); bm2 = np.asarray(inputs["bm2"], np.float32)
    Wu1 = np.asarray(inputs["Wu1"], np.float32); bu1 = np.asarray(inputs["bu1"], np.float32)
    Wu2 = np.asarray(inputs["Wu2"], np.float32); bu2 = np.asarray(inputs["bu2"], np.float32)

    N, Hh = nemb.shape
    G = gpos.shape[0]
    E = eidx.shape[1]
    B = frames.shape[0]
    assert Hh == H

    i_all = eidx[0]
    jg_all = eidx[1] - N
    order = np.argsort(jg_all, kind="stable")
    jg_s = jg_all[order]
    i_s = i_all[order]

    gb = jg_s // BLK
    counts_blk = np.bincount(gb, minlength=G // BLK)
    bmix = bm1 + be2 @ Wm1[H:]
    has_bmix = bool(np.any(np.abs(bmix) > 0))

    # rank-matched variable tiles-per-block: each core sorts its own
    # blocks by size so the shared per-rank tile counts are minimal
    nblk = (G // ncores) // BLK
    chunk_tiles = 32
    tb = np.ceil(counts_blk / P).astype(np.int64).reshape(ncores, nblk)
    bord = np.argsort(-tb, axis=1, kind="stable")    # [ncores, nblk]
    sorted_tb = np.take_along_axis(tb, bord, axis=1)
    shared = sorted_tb.max(axis=0)                   # [nblk]
    shared[-1] += (-shared.sum()) % chunk_tiles      # pad to chunk multiple
    cfg = Cfg(N, G, E, B, ncores, shared.tolist(), has_bmix=has_bmix)
    cfg.bord = bord                                  # for output reassembly

    inv_ord = np.empty_like(bord)
    np.put_along_axis(inv_ord, bord, np.arange(nblk)[None, :], axis=1)
    tile_off = np.zeros(nblk + 1, np.int64)
    tile_off[1:] = np.cumsum(shared)

    # destination slot for each sorted edge
    starts = np.zeros(G // BLK + 1, np.int64)
    starts[1:] = np.cumsum(counts_blk)
    rank = np.arange(E) - starts[gb]
    core_e = gb // cfg.nblk
    b_local = gb % cfg.nblk
    prog_i = inv_ord[core_e, b_local]                # program-block index
    slot = tile_off[prog_i] * P + rank

    # per-edge host gathers (pure data movement + O(N)/O(B) math)
    R_flat = frames.reshape(B, 9)
    b_e = batch[i_s]
    z_r = R_flat[b_e].T.astype(np.float32)               # [9, E]
    gp_e = gpos[jg_s].T.astype(np.float32)               # [3, E]
    gp_rep = np.tile(gp_e, (3, 1))                       # [9, E]
    rp_node = np.einsum("nab,nb->na", frames[batch], npos).astype(np.float32)
    rp_e = rp_node[i_s].T                                # [3, E]

    ecount = np.bincount(jg_all, minlength=G).astype(np.float32)
    nemb_bf = nemb.astype(ml_dtypes.bfloat16)

    # shared constant tensors
    shared_consts = {
        "W1bigb": np.ascontiguousarray(
            np.concatenate([np.repeat(We1, 3, axis=0), -We1], 0)
        ).astype(ml_dtypes.bfloat16),
        "be1c": np.ascontiguousarray(be1[:, None]),
        "Wfb": np.ascontiguousarray(We2 @ Wm1[H:]).astype(ml_dtypes.bfloat16),
        "Wm1topb": np.ascontiguousarray(Wm1[:H]).astype(ml_dtypes.bfloat16),
        "I128": np.eye(P, dtype=np.float32),
        "IOTA": np.ascontiguousarray(
            np.tile(np.arange(P, dtype=np.float32)[None, :], (P, 1))
        ).astype(ml_dtypes.bfloat16),
        "Wm2": np.ascontiguousarray(Wm2),
        "bm2r": np.ascontiguousarray(bm2[None, :]),
        "Wu1": np.ascontiguousarray(Wu1),
        "bu1c": np.ascontiguousarray(bu1[:, None]),
        "Wu2": np.ascontiguousarray(Wu2),
        "bu2c": np.ascontiguousarray(bu2[:, None]),
    }
    if has_bmix:
        shared_consts["bmix4"] = np.ascontiguousarray(
            np.tile(bmix[None, :], (P, 4)))

    in_maps = []
    for c in range(ncores):
        sel = core_e == c
        sl = slot[sel]
        zin = np.zeros((21, cfg.e_pad), np.float32)
        zin[0:9, sl] = gp_rep[:, sel]
        zin[9:18, sl] = z_r[:, sel]
        zin[18:21, sl] = rp_e[:, sel]
        embT = np.zeros((cfg.e_pad, H), ml_dtypes.bfloat16)
        embT[sl] = nemb_bf[i_s[sel]]
        segf_f = np.full(cfg.e_pad, -1.0, np.float32)
        segf_f[sl] = (jg_s[sel] % BLK).astype(np.float32)

        segf_t = segf_f.reshape(cfg.ntiles, P).T            # [128, ntiles]
        cnt_c = ecount[c * cfg.segs_core:(c + 1) * cfg.segs_core]
        cnt_prog = cnt_c.reshape(cfg.nblk, P)[bord[c]]      # program order
        rinv_t = (1.0 / np.maximum(cnt_prog, 1.0)).T \
            .astype(np.float32)                             # [128, nblk]

        m = dict(shared_consts)
        m["zin"] = np.ascontiguousarray(zin.astype(ml_dtypes.bfloat16))
        m["nembET"] = np.ascontiguousarray(embT.T)
        m["segf"] = np.ascontiguousarray(segf_t)
        m["rinv"] = np.ascontiguousarray(rinv_t)
        m["gates"] = np.ascontiguousarray(
            np.minimum(cnt_prog, 1.0).reshape(1, -1).astype(np.float32))
        in_maps.append(m)

    const_row = silu_np(bu1) @ Wu2 + bu2
    return cfg, in_maps, const_row


def assemble_output(cfg, results, const_row, N, G):
    out = np.empty((N + G, H), np.float32)
    out[:N] = const_row[None, :]
    for c in range(cfg.ncores):
        oc = results[c]["outT"].T.reshape(cfg.nblk, BLK, H)
        base = N + c * cfg.segs_core
        for i in range(cfg.nblk):
            b = cfg.bord[c, i]
            out[base + b * BLK: base + (b + 1) * BLK] = oc[i]
    return out


# ======================= top-level kernel entry =======================

_PROGRAM_CACHE = {}

NCORES = 8


def kernel(**inputs):
    """Full-input entry point: shards edges by destination grid cell across
    8 NeuronCores, runs the Bass/Tile program, reassembles the full output."""
    from concourse.bass_utils import run_bass_kernel_spmd

    cfg, in_maps, const_row = host_prep(inputs, NCORES)
    key = (cfg.tiles_per_block, cfg.has_bmix)
    if key not in _PROGRAM_CACHE:
        _PROGRAM_CACHE[key] = build_program(cfg)
    nc = _PROGRAM_CACHE[key]
    res = run_bass_kernel_spmd(nc, in_maps, core_ids=list(range(NCORES)))
    N = inputs["node_pos"].shape[0]
    G = inputs["grid_pos"].shape[0]
    return assemble_output(cfg, res.results, const_row, N, G)
